# revision 45
# baseline (speedup 1.0000x reference)
"""Trainium2 Bass kernel for nn_LossSoftDice (soft-dice loss over 32 samples
of 1x512x512 probability/target maps).

Strategy: pure data parallel over the batch. Each of the 8 NeuronCores gets 4
samples (each sample = 262144 f32 elements, viewed as a [128, 2048] tile).
The device computes only per-sample / per-partition statistics; the host
combine (the gather/unshard step) does the O(P) reductions and the exact
acc-branch correction scan, identical in structure to the original version.

Final layout (per core, 4 samples, md tile = [128, 4096] = [m2 | m1]):
  - Input DMA: 8 full tiles of [128, 2048] (1 MiB, 8KB rows — the only
    shape that sustains ~215 GB/s per HWDGE queue; smaller descriptors
    halve the per-queue rate).  All targets ride the sync queue, all probs
    the scalar queue (~430 GB/s combined), so m2_k lands ~2.5us before
    m1_k — exactly the stagger the per-sample max -> stt DVE chain wants.
  - DVE (critical engine, ~18.3us): maxp (tensor_reduce max) + inter
    (scalar_tensor_tensor m1*m2 + accum) per sample.
  - ACT: 4 probs DMA issues, nsr via Sign(m1 - 0.5) + accum (sum of +-1s;
    host recovers the count as (N + sum)/2), and the per-sample den
    readout (Copy + accum of the psum row).
  - PE (otherwise idle): den_b = sum(m1)+sum(m2) via 8 accumulating fp32r
    matmuls [128,512] x ones[128,1] -> psum[1,512] per sample.
  - Teardown: drain-only (no final barrier / sem-clear storm) and the ACT
    table load is re-placed after the scalar queue's DMA issues.

Host combine (exact, matches the reference's acc branch):
  gmax = max_p maxp[p];  corr = N - nSR - K + 2A, where K (#elements equal to
  gmax) and A (#those with m1 > 0.5) come from scanning only the partitions
  whose maxp equals gmax (O(2048) per sample against the host-held inputs).
  score = 2*(inter+1)/(den+1);  score = 1 where corr == 1;
  loss = mean(1 - score)
"""

import os
import sys
import types

import numpy as np


def _ensure_concourse():
    try:
        import concourse.bass  # noqa: F401
    except ImportError:
        for p in ("/opt/trn_rl_repo", "/root/.axon_site/_ro/trn_rl_repo"):
            if os.path.isdir(p) and p not in sys.path:
                sys.path.insert(0, p)
        import concourse.bass  # noqa: F401


_ensure_concourse()

import concourse.bass as bass  # noqa: E402
import concourse.bacc as bacc  # noqa: E402
import concourse.tile as tile  # noqa: E402
from concourse import mybir  # noqa: E402
from concourse.bass_utils import run_bass_kernel_spmd  # noqa: E402
from concourse.vector_clock import ScopedClock  # noqa: E402

N_CORES = 8
B = 32                      # total batch
BPC = B // N_CORES          # samples per core
P = 128                     # partitions
F = 2048                    # free dim per partition (P*F = 512*512)

# stats tile columns
C_INTER = 0                 # cols 0..3   : per-partition sum m1*m2
C_NSR = 4                   # cols 4..7   : per-partition sum sign(m1-0.5)
C_MAX = 8                   # cols 8..11  : per-partition max m2
C_DEN = 12                  # cols 12..15 : row 0 only: sum(m1)+sum(m2)
NCOLS = 16


def _slim_drain_and_barrier(self, tick_clock, wait_clock):
    # TileContext._drain_and_barrier without the trailing all-engine
    # barrier and the per-sem clear storm (~4us on the vector engine): the
    # NEFF is executed exactly once per process here, so semaphore state
    # does not need to be restored for a re-execution.  The drain still
    # waits for every tracked semaphore (all DMAs + compute) to land.
    nc = self.nc
    drain_inst = nc.sync.drain()
    wait_clock.add_sem_waits(
        drain_inst.ins, ScopedClock({None: tick_clock.global_clock})
    )
    assert self.sems is not None
    popped = nc._tile_sem_poison_stack.pop()
    assert popped is self._sem_poison
    for sem in self.sems.allocated().values():
        nc.release_semaphore(sem)


tile.TileContext._drain_and_barrier = _slim_drain_and_barrier


def _install_ntff_hook_module():
    """bass_utils imports antenv.axon_hooks when trace=True under axon; this
    container's antenv lacks that module. Recreate it from the boot helper."""
    if "antenv.axon_hooks" in sys.modules:
        return
    try:
        import trn_agent_boot.trn_boot as tb

        hook = tb._ntff_profile_via_ctypes("/opt/axon/libaxon_pjrt.so")
    except Exception:
        hook = None
    m = types.ModuleType("antenv.axon_hooks")
    m.get_axon_ntff_profile_hook = lambda: hook
    m.set_axon_ntff_profile_hook = lambda h: None
    sys.modules["antenv.axon_hooks"] = m


def _build_nc():
    nc = bacc.Bacc("TRN2", debug=False)
    f32 = mybir.dt.float32
    f32r = mybir.dt.float32r
    probs = nc.dram_tensor("probs", [BPC, P, F], f32, kind="ExternalInput").ap()
    targets = nc.dram_tensor("targets", [BPC, P, F], f32, kind="ExternalInput").ap()
    stats_out = nc.dram_tensor("stats", [P, NCOLS], f32, kind="ExternalOutput").ap()

    A = mybir.AluOpType
    AF = mybir.ActivationFunctionType
    with tile.TileContext(nc) as tc:
        with (
            tc.tile_pool(name="md", bufs=BPC) as md_pool,
            tc.tile_pool(name="scr", bufs=1) as scr_pool,
            tc.tile_pool(name="stats", bufs=1) as stats_pool,
            tc.psum_pool(name="pp", bufs=BPC) as psum_pool,
        ):
            # f32r memset is illegal ISA: memset an f32 ones vector, then a
            # one-element DVE copy produces the f32r-typed view the verifier
            # accepts as an fp32r producer for the matmuls.
            ones_f = scr_pool.tile([P, 1], f32, tag="ones_f")
            nc.gpsimd.memset(ones_f[:], 1.0)
            neg_half = scr_pool.tile([P, 1], f32, tag="neg_half")
            nc.gpsimd.memset(neg_half[:], -0.5)
            st = stats_pool.tile([P, NCOLS], f32, tag="st", name="st_all")
            nc.gpsimd.memset(st[:], 0.0)
            ones_t = scr_pool.tile([P, 1], f32r, tag="ones")
            nc.vector.tensor_scalar(
                ones_t[:], ones_f[:], 0.0, None, mybir.AluOpType.add
            )

            mds = [
                md_pool.tile([P, 2 * F], f32, tag="md", name=f"md{s}")
                for s in range(BPC)
            ]
            # 8 full-tile DMAs of [128, 2048] (1 MiB, 8KB rows — the shape
            # that sustains ~215 GB/s per queue; smaller descriptors halve
            # the per-queue rate).  All targets ride the sync queue and all
            # probs the scalar queue: both queues then deliver sample k at a
            # ~4.7us cadence, with m2_k landing ~2.3us before m1_k (the
            # scalar queue starts late behind the ACT table load), which is
            # exactly the stagger the per-sample max -> stt DVE chain wants.
            # The tiles are DMA'd as float32r views (byte-identical) so the
            # BIR verifier accepts the fp32r matmuls that consume them.
            for s in range(BPC):
                md = mds[s]
                nc.sync.dma_start(
                    md[:, 0:F].bitcast(f32r), targets[s].bitcast(f32r)
                )
                nc.scalar.dma_start(
                    md[:, F : 2 * F].bitcast(f32r), probs[s].bitcast(f32r)
                )

            # Pre-place the ACT table load (set 0 covers Sign + Copy) AFTER
            # the scalar engine's DMA issues: Bacc's insert_act_table_loads
            # pass then sees every activation covered and skips its own
            # block-entry insertion, which otherwise delays the probs queue
            # start by the ~1.6us table-load time.
            nc.scalar.add_instruction(
                mybir.InstLoadActFuncSet(
                    name=nc.get_next_instruction_name(),
                    act_func_set_id=0,
                    ins=[],
                    outs=[],
                )
            )

            dve_scr = scr_pool.tile([P, F], f32, tag="dve_scr")
            sign_scr = scr_pool.tile([P, F], f32, tag="sign_scr")
            den_scr = scr_pool.tile([1, 512], f32, tag="den_scr")
            pbanks = [
                psum_pool.tile([P, 512], f32, tag="pb", name=f"pb{s}")
                for s in range(BPC)
            ]

            ones_r = ones_t[:]

            def emit_max(s):
                nc.vector.tensor_reduce(
                    st[:, C_MAX + s : C_MAX + s + 1],
                    mds[s][:, 0:F],
                    mybir.AxisListType.X,
                    A.max,
                )

            def emit_stt(s):
                nc.vector.scalar_tensor_tensor(
                    out=dve_scr[:],
                    in0=mds[s][:, F : 2 * F],
                    scalar=1.0,
                    in1=mds[s][:, 0:F],
                    op0=A.mult,
                    op1=A.mult,
                    accum_out=st[:, C_INTER + s : C_INTER + s + 1],
                )

            def emit_sign(s):
                nc.scalar.activation(
                    sign_scr[:],
                    mds[s][:, F : 2 * F],
                    AF.Sign,
                    bias=neg_half[:],
                    accum_out=st[:, C_NSR + s : C_NSR + s + 1],
                )

            def emit_matmuls(s):
                for j in range(8):
                    nc.tensor.matmul(
                        pbanks[s][0:1, :],
                        ones_r,
                        mds[s][:, j * 512 : (j + 1) * 512].bitcast(f32r),
                        start=(j == 0),
                        stop=(j == 7),
                    )

            def emit_den(s):
                nc.scalar.activation(
                    den_scr[0:1, :],
                    pbanks[s][0:1, :],
                    AF.Copy,
                    accum_out=st[0:1, C_DEN + s : C_DEN + s + 1],
                )

            # Per sample: DVE max then stt (4.57us, just under the 4.66us DMA
            # cadence), ACT Sign + den readout, PE 8 accumulating matmuls.
            for s in range(BPC):
                emit_max(s)
                emit_stt(s)
                emit_sign(s)
                emit_matmuls(s)
                if s == BPC - 1:
                    # Issue the bulk stats DMA (inter/nsr/max, 132 rows of
                    # descriptors) before the final den readout: it only
                    # needs the last DVE accumulator read, so it overlaps
                    # the PE->ACT den3 chain.  Stats ride the scalar queue,
                    # still warm from delivering m1_3 (the idle sync queue
                    # pays a ~1.5us re-arm latency).
                    nc.scalar.dma_start(
                        stats_out[:, 0:C_DEN], st[:, 0:C_DEN]
                    )
                emit_den(s)

            # Only this 16-byte single-descriptor DMA trails den3.
            nc.scalar.dma_start(
                stats_out[0:1, C_DEN:NCOLS], st[0:1, C_DEN:NCOLS]
            )

    nc.compile()
    # Bacc's insert_act_table_loads still adds its own block-entry load in
    # addition to our pre-placed one; drop the earlier duplicate so the
    # scalar engine issues its DMAs before the ~1.6us table load.  The auto
    # load is inserted after sem legalization and carries no sync info, so
    # deleting it is safe.
    for b in nc.main_func.blocks:
        loads = [
            i
            for i, inst in enumerate(b.instructions)
            if isinstance(inst, mybir.InstLoadActFuncSet)
        ]
        if len(loads) > 1:
            si = b.instructions[loads[0]].sync_info
            assert si is None or (not si.on_wait and not si.on_update), si
            del b.instructions[loads[0]]
    return nc


def _shard_inputs(probs, targets):
    probs = np.ascontiguousarray(np.asarray(probs, dtype=np.float32)).reshape(B, P, F)
    targets = np.ascontiguousarray(np.asarray(targets, dtype=np.float32)).reshape(
        B, P, F
    )
    in_maps = []
    for i in range(N_CORES):
        sl = slice(i * BPC, (i + 1) * BPC)
        in_maps.append(
            {
                "probs": np.ascontiguousarray(probs[sl]),
                "targets": np.ascontiguousarray(targets[sl]),
            }
        )
    return in_maps


def _combine(results, probs, targets):
    """Exact host-side combine of per-partition stats -> scalar loss.

    corr_b = N - nSR - K + 2A with K (#elements == global max) and
    A (#those with m1 > 0.5) recovered by scanning only the partitions
    that attain the global max (O(2048) per sample, exact)."""
    inter = np.empty(B)
    den = np.empty(B)
    corr = np.empty(B)
    N = float(P * F)
    for i in range(N_CORES):
        r = results[i]["stats"]
        for s in range(BPC):
            b = i * BPC + s
            inter[b] = r[:, C_INTER + s].astype(np.float64).sum()
            den[b] = float(r[0, C_DEN + s])
            sgn = r[:, C_NSR + s].astype(np.float64).sum()
            if np.count_nonzero(probs[b] == 0.5):
                # Sign(0) semantics are ambiguous; count exactly on host.
                nsr = float(np.count_nonzero(probs[b] > 0.5))
            else:
                nsr = (sgn + N) / 2.0  # exact: sum of +-1 per element
            maxp = r[:, C_MAX + s]
            gmax = maxp.max()
            K = Acnt = 0
            for p in np.nonzero(maxp == gmax)[0]:
                hit = targets[b, p, :] == gmax
                K += int(hit.sum())
                Acnt += int((hit & (probs[b, p, :] > 0.5)).sum())
            corr[b] = N - nsr - K + 2 * Acnt
    score = 2.0 * (inter + 1.0) / (den + 1.0)
    score = np.where(corr == 1.0, 1.0, score)
    return np.array(np.mean(1.0 - score), dtype=np.float32)


def _run(probs, targets, trace=False, tmpdir=None):
    _install_ntff_hook_module()
    nc = _build_nc()
    in_maps = _shard_inputs(probs, targets)
    res = run_bass_kernel_spmd(
        nc, in_maps, list(range(N_CORES)), trace=trace, tmpdir=tmpdir
    )
    pr = np.asarray(probs, dtype=np.float32).reshape(B, P, F)
    tg = np.asarray(targets, dtype=np.float32).reshape(B, P, F)
    out = _combine(res.results, pr, tg)
    return out, res


def kernel(probs, targets):
    out, _ = _run(probs, targets)
    return out
